# revision 95
# baseline (speedup 1.0000x reference)
"""BiMamba layer Trainium2 kernel (8 NeuronCores, SPMD).

Sharding: 4 batch-groups x 2 d_inner-halves. Core (g, h) handles the 3
(b*f) scan units of batch g for d_inner channels G=[96h, 96h+96), both
scan directions. Each core emits a partial out-projection; the host sums
the two halves per batch and adds out_proj_b.

Key structure (all bf16 matmuls / elementwise, f32 PSUM):
  - conv+in_proj fused via 9 shifted matmuls; conv output channels are
    PERMUTED per core so the core's own 96 channels come first -> the
    SSM input u is a view xc0[0:96], no separate projection.
  - delta pre-activation matrix composed on host: Wd = dt_proj_w[G] @
    x_proj_dt  (96x192), so delta = softplus(Wd @ xc) directly
    (softplus via Exp then Ln(1+x), both in one act table).
  - B/C are produced directly in n-replicated 128-row form by repeating
    x_proj rows (wxpbc). crep_rev is a reversed AP view, not computed.
  - per k-tile (8 d-channels x 16 n-states = 128 rows):
      delta/du replicated 8->128 by SP-queue DMA broadcast (tail ks) or
      a small selection matmul (head KP ks); daf = Exp(A * delta_rep) on
      ACT; both scans on DVE (TensorTensorScanArith is DVE-only on TRN2);
      dbu and the hc mults of POOL_HC*_KS ks on GPSIMD (Pool), remaining
      hc mults on DVE; n-reduction via per-k seln matmuls accumulating
      into unit-lifetime [96, 512] PSUM tiles (start at k=0, stop k=11).
  - 2*D*u term folded into the z-half of out_proj weights on host.
  - software pipeline: staged k-loop (produce k / scan k-1 / reduce k-2)
    with the next unit's front-end emitted mid-loop; PE warmup matmuls
    cover the pstate ramp during the initial xt DMA.
"""
from contextlib import ExitStack

import numpy as np

import concourse.bass as bass
import concourse.tile as tile
from concourse import bacc, mybir
from concourse.bass_utils import run_bass_kernel_spmd

F32 = mybir.dt.float32
BF = mybir.dt.bfloat16
AF = mybir.ActivationFunctionType
OP = mybir.AluOpType

B, SEQ, DIM = 4, 6144, 384
L = 2048                  # per-unit sequence length
NU = 3                    # units per core
DIN, DH, NST, DTR = 192, 96, 16, 24
NK = 12                   # (DH*NST)//128 row-tiles
LC = 512                  # psum column chunk
NLC = L // LC
NCORES = 8
KP = 2                    # head k-tiles using the PE selection path
# scans are DVE-only on TRN2 hardware; Pool (GPSIMD) takes the dbu
# multiplies and part of the h*C multiplies to offload DVE
POOL_HCF_KS = (5, 9)
POOL_HCB_KS = (2, 5, 9)   # k-tiles whose hc mults run on Pool

_NC_CACHE = {}


def _build(ab_same: bool, debug: bool = False):
    nc = bacc.Bacc("TRN2", target_bir_lowering=False, debug=False)

    def din(name, shape, dt=F32):
        return nc.dram_tensor(name, list(shape), dt, kind="ExternalInput").ap()

    xtp_d = din("xtp", (NU, 3, 128, L + 2), BF)
    wm_d = din("wm", (9, 128, DIN), BF)
    efix_d = din("efix", (1, 2, DIN), BF)
    one_d = din("one", (1, 1), BF)
    bsil_d = din("bsil", (128, 2))
    wd1_d = din("wd1", (128, DH), BF)
    wd2_d = din("wd2", (64, DH), BF)
    wxpbc1_d = din("wxpbc1", (128, 256), BF)
    wxpbc2_d = din("wxpbc2", (64, 256), BF)
    bsp_d = din("bsp", (DH, 1))
    acol_d = din("acol", (128, NK))
    abcol_d = din("abcol", (128, NK))
    seli_d = din("seli", (DH, KP * 128), BF)
    seln_d = din("seln", (NK, 128, DH), BF)
    wouty_d = din("wouty", (DH, DIM), BF)
    woutz_d = din("woutz", (DH, DIM), BF)
    out_d = nc.dram_tensor("out", [NU, L, DIM], F32, kind="ExternalOutput").ap()
    dbg = {}
    if debug:
        for name, shape, dt_ in [
                ("dbg_xc0", (128, L), BF), ("dbg_xc1", (64, L), BF),
                ("dbg_delta", (DH, L), BF), ("dbg_du", (DH, L), BF),
                ("dbg_brep", (128, L), BF), ("dbg_crep", (128, L), BF),
                ("dbg_daf", (128, L), BF), ("dbg_dbu", (128, L), BF),
                ("dbg_hf", (128, L), BF), ("dbg_y", (DH, L), BF)]:
            dbg[name] = nc.dram_tensor(name, list(shape), dt_,
                                       kind="ExternalOutput").ap()

    hbufs = 3 if ab_same else 2
    with tile.TileContext(nc) as tc, ExitStack() as ctx:
        cp = ctx.enter_context(tc.tile_pool(name="consts", bufs=1))
        px = ctx.enter_context(tc.tile_pool(name="px", bufs=2))
        pxc = ctx.enter_context(tc.tile_pool(name="pxc", bufs=2))
        psm = ctx.enter_context(tc.tile_pool(name="psm", bufs=2))
        pesp = ctx.enter_context(tc.tile_pool(name="pesp", bufs=1))
        pbig = ctx.enter_context(tc.tile_pool(name="pbig", bufs=2))
        pout = ctx.enter_context(tc.tile_pool(name="pout", bufs=4))
        ppa = ctx.enter_context(tc.tile_pool(name="ppa", bufs=2, space="PSUM"))
        ppd = ctx.enter_context(tc.tile_pool(name="ppd", bufs=2, space="PSUM"))
        ppy = ctx.enter_context(tc.tile_pool(name="ppy", bufs=1, space="PSUM"))

        # per-unit persistent tiles, double buffered via pool rotation
        (xts, xc0s, xc1s, breps, creps, deltas, dus, esps) = (
            {} for _ in range(8))

        def emit_xt_dma(u):
            # column-chunked load: conv of chunk lc only needs cols
            # [lc*LC, lc*LC+LC+2), so the first chunk unblocks conv fast
            xt = px.tile([128, 3, L + 2], BF, name="xt")
            xts[u] = xt
            for lc in range(NLC):
                c0 = lc * LC
                cw = LC + 2 if lc == NLC - 1 else LC
                nc.sync.dma_start(xt[:, :, c0:c0 + cw],
                                  xtp_d[u][:, :, c0:c0 + cw]
                                  .transpose([1, 0, 2]))

        # ---- startup-critical DMAs first: conv weights, first xt(0)
        # column-chunk, silu bias, then the rest of xt(0)
        wm_sb = cp.tile([128, 9, DIN], BF)
        nc.sync.dma_start(wm_sb[:], wm_d.transpose([1, 0, 2]))
        xt0 = px.tile([128, 3, L + 2], BF, name="xt")
        xts[0] = xt0
        nc.sync.dma_start(xt0[:, :, 0:LC],
                          xtp_d[0][:, :, 0:LC].transpose([1, 0, 2]))
        bsil_sb = cp.tile([128, 2], F32)
        nc.sync.dma_start(bsil_sb[:], bsil_d)
        efix_sb = cp.tile([1, 2, DIN], BF)
        nc.sync.dma_start(efix_sb[:], efix_d)
        one_sb = cp.tile([1, 1], BF)
        nc.sync.dma_start(one_sb[:], one_d)
        for lc in range(1, NLC):
            c0 = lc * LC
            cw = LC + 2 if lc == NLC - 1 else LC
            nc.sync.dma_start(xt0[:, :, c0:c0 + cw],
                              xtp_d[0][:, :, c0:c0 + cw].transpose([1, 0, 2]))

        # PE warmup: dummy matmuls on the first-loaded constant keep the
        # tensor engine's pstate ramp going while xt(0) streams in, so the
        # first conv matmuls run at full clock instead of cold/mid rate.
        warm = ppd.tile([128, LC], F32, tag="ppd", name="ps_warm")
        for _ in range(21):
            nc.tensor.matmul(warm[:, 0:DIN], wm_sb[:, 0, 0:128],
                             wm_sb[:, 0, 0:DIN], start=True, stop=True)

        def emit_fe_conv(u, lcs=None):
            """conv+in_proj+silu -> xc0/xc1 (optionally only some lc)."""
            xt = xts[u]
            if lcs is None or 0 in lcs:
                xc0 = pxc.tile([128, L], BF, name="xc0")
                xc1 = pxc.tile([64, L], BF, name="xc1")
                xc0s[u], xc1s[u] = xc0, xc1
            xc0, xc1 = xc0s[u], xc1s[u]
            # conv groups (all silu acts contiguous -> one table region)
            for lc in (range(NLC) if lcs is None else lcs):
                for c0, cw, dst, bias_ap in (
                        (0, 128, xc0, bsil_sb[0:128, 0:1]),
                        (128, 64, xc1, bsil_sb[0:64, 1:2])):
                    ps = ppa.tile([128, LC], F32, tag="ppa", name="ps_conv")
                    mms = []
                    for s in range(3):
                        for kt in range(3):
                            mms.append((ps[0:cw, :],
                                        wm_sb[:, s * 3 + kt, c0:c0 + cw],
                                        xt[:, kt, s + lc * LC:s + lc * LC + LC]))
                    if lc == 0:
                        mms.append((ps[0:cw, 0:1],
                                    efix_sb[0:1, 0, c0:c0 + cw], one_sb[:]))
                    if lc == NLC - 1:
                        mms.append((ps[0:cw, LC - 1:LC],
                                    efix_sb[0:1, 1, c0:c0 + cw], one_sb[:]))
                    for i, (o, lh, rh) in enumerate(mms):
                        nc.tensor.matmul(o, lh, rh, start=(i == 0),
                                         stop=(i == len(mms) - 1))
                    nc.scalar.activation(dst[:, lc * LC:(lc + 1) * LC],
                                         ps[0:cw, :], AF.Silu, bias=bias_ap)

        def emit_fe_xproj(u, first=False, halves_sel=(0, 1)):
            """x_proj -> brep/crep; delta = softplus(Wd@xc); du = delta*u."""
            xc0, xc1 = xc0s[u], xc1s[u]
            if 0 in halves_sel:
                brep = psm.tile([128, L], BF, name="brep")
                crep = psm.tile([128, L], BF, name="crep")
                delta = psm.tile([DH, L], BF, name="delta")
                du = psm.tile([DH, L], BF, name="du")
                breps[u], creps[u] = brep, crep
                deltas[u], dus[u] = delta, du
            brep, crep = breps[u], creps[u]
            delta, du = deltas[u], dus[u]
            # delta = softplus(Wd@xc + b) via Exp then Ln(1+x).  For the
            # first unit Ln + du run per L-half to unblock the scans early;
            # later units batch them (fewer act-table switches).
            if 0 in halves_sel:
                esps[u] = pesp.tile([DH, L], BF, name="esp")
            esp = esps[u]
            halves = 2 if first else 1
            HW = L // halves
            for half in (halves_sel if first else (0,)):
                for lc in range(half * NLC // halves,
                                (half + 1) * NLC // halves):
                    sl = slice(lc * LC, (lc + 1) * LC)
                    pdp = ppa.tile([128, LC], F32, tag="ppa", name="ps_dp")
                    nc.tensor.matmul(pdp[0:DH, :], wd1_sb[:], xc0[:, sl],
                                     start=True, stop=False)
                    nc.tensor.matmul(pdp[0:DH, :], wd2_sb[:], xc1[:, sl],
                                     start=False, stop=True)
                    nc.scalar.activation(esp[:, sl], pdp[0:DH, :], AF.Exp,
                                         bias=bsp_sb[:])
                hs = slice(half * HW, (half + 1) * HW)
                nc.scalar.activation(delta[:, hs], esp[:, hs], AF.Ln,
                                     bias=1.0)
                nc.vector.tensor_tensor(du[:, hs], delta[:, hs],
                                        xc0[0:DH, hs], OP.mult)
            # x_proj (B/C replicated)
            for lc in ([h2 * 2 + j for h2 in halves_sel for j in (0, 1)]
                       if first else range(NLC)):
                sl = slice(lc * LC, (lc + 1) * LC)
                pbr = ppa.tile([128, LC], F32, tag="ppa", name="ps_br")
                nc.tensor.matmul(pbr[:], wxpbc1_sb[:, 0:128], xc0[:, sl],
                                 start=True, stop=False)
                nc.tensor.matmul(pbr[:], wxpbc2_sb[:, 0:128], xc1[:, sl],
                                 start=False, stop=True)
                nc.scalar.activation(brep[:, sl], pbr[:], AF.Copy)
                pcr = ppa.tile([128, LC], F32, tag="ppa", name="ps_cr")
                nc.tensor.matmul(pcr[:], wxpbc1_sb[:, 128:256], xc0[:, sl],
                                 start=True, stop=False)
                nc.tensor.matmul(pcr[:], wxpbc2_sb[:, 128:256], xc1[:, sl],
                                 start=False, stop=True)
                nc.scalar.activation(crep[:, sl], pcr[:], AF.Copy)
            if debug and u == 0:
                nc.sync.dma_start(dbg["dbg_xc0"], xc0[:])
                nc.sync.dma_start(dbg["dbg_xc1"], xc1[:])
                nc.sync.dma_start(dbg["dbg_delta"], delta[:])
                nc.sync.dma_start(dbg["dbg_du"], du[:])
                nc.sync.dma_start(dbg["dbg_brep"], brep[:])
                nc.sync.dma_start(dbg["dbg_crep"], crep[:])

        def emit_stage_a(u, k, st):
            """produce durep, daf, dbu for tile k."""
            delta, du = deltas[u], dus[u]
            brep = breps[u]
            durep = pbig.tile([128, L], BF, name="t_durep", bufs=3)
            nc.sync.dma_start(
                durep[:],
                du[8 * k:8 * k + 8, :].unsqueeze(1).broadcast_to([8, 16, L]))
            daf = pbig.tile([128, L], BF, name="t_daf", bufs=3)
            dab = daf if ab_same else pbig.tile([128, L], BF, name="t_dab",
                                                bufs=3)
            if k < KP:
                for lc in range(NLC):
                    sl = slice(lc * LC, (lc + 1) * LC)
                    pd = ppd.tile([128, LC], F32, tag="ppd", name="ps_sel")
                    nc.tensor.matmul(pd[:], seli_sb[:, 128 * k:128 * (k + 1)],
                                     delta[:, sl], start=True, stop=True)
                    nc.scalar.activation(daf[:, sl], pd[:], AF.Exp,
                                         scale=acol_sb[:, k:k + 1])
                    if not ab_same:
                        nc.scalar.activation(dab[:, sl], pd[:], AF.Exp,
                                             scale=abcol_sb[:, k:k + 1])
            else:
                drep = pbig.tile([128, L], BF, name="t_drep", bufs=3)
                nc.sync.dma_start(
                    drep[:],
                    delta[8 * k:8 * k + 8, :].unsqueeze(1)
                    .broadcast_to([8, 16, L]))
                nc.scalar.activation(daf[:], drep[:], AF.Exp,
                                     scale=acol_sb[:, k:k + 1])
                if not ab_same:
                    nc.scalar.activation(dab[:], drep[:], AF.Exp,
                                         scale=abcol_sb[:, k:k + 1])
            dbu = pbig.tile([128, L], BF, name="t_dbu", bufs=3)
            # during the cold start the Pool->DVE hop costs more than DVE
            # doing the multiply itself
            eng_d = nc.vector if (u == 0 and k < 1) else nc.gpsimd
            eng_d.tensor_tensor(dbu[:], durep[:], brep[:], OP.mult)
            st[k] = dict(daf=daf, dab=dab, dbu=dbu)

        def emit_stage_b(u, k, st):
            """the two scans for tile k."""
            daf, dab, dbu = st[k]["daf"], st[k]["dab"], st[k]["dbu"]
            hf = pbig.tile([128, L], BF, name="t_hf", bufs=hbufs)
            hb = pbig.tile([128, L], BF, name="t_hb", bufs=hbufs)
            nc.vector.tensor_tensor_scan(hf[:], daf[:], dbu[:], 0.0,
                                         OP.mult, OP.add)
            nc.vector.tensor_tensor_scan(hb[:], dab[:, ::-1], dbu[:, ::-1],
                                         0.0, OP.mult, OP.add)
            st[k]["hf"], st[k]["hb"] = hf, hb

        def emit_stage_c(u, k, st, pys):
            """hc mults + n-reduction matmuls for tile k."""
            crep = creps[u]
            hf, hb = st[k]["hf"], st[k]["hb"]
            eng_f = nc.gpsimd if k in POOL_HCF_KS else nc.vector
            eng_b = nc.gpsimd if k in POOL_HCB_KS else nc.vector
            hcf = pbig.tile([128, L], BF, name="t_hcf", bufs=hbufs + 1)
            eng_f.tensor_tensor(hcf[:], hf[:], crep[:], OP.mult)
            hcb = pbig.tile([128, L], BF, name="t_hcb", bufs=hbufs + 1)
            eng_b.tensor_tensor(hcb[:], hb[:], crep[:, ::-1], OP.mult)
            del st[k]
            for lc in range(NLC):
                sl = slice(lc * LC, (lc + 1) * LC)
                nc.tensor.matmul(pys[lc][:], seln_sb[:, k, :],
                                 hcf[:, sl], start=(k == 0), stop=False)
                nc.tensor.matmul(pys[lc][:], seln_sb[:, k, :],
                                 hcb[:, ::-1][:, sl], start=False,
                                 stop=(k == NK - 1))

        def emit_scan_tail(u, pys, last=False):
            """y copy + out_proj partial + store.  For the last unit the
            po tiles alternate between the ppa and (then idle) ppd pools so
            the PE/Act drain chain pipelines twice as deep."""
            xc0 = xc0s[u]
            y_sb = psm.tile([DH, L], BF, name="y_sb")
            for lc in range(NLC):
                sl = slice(lc * LC, (lc + 1) * LC)
                nc.scalar.activation(y_sb[:, sl], pys[lc][:], AF.Copy)
            if debug and u == 0:
                nc.sync.dma_start(dbg["dbg_y"], y_sb[:])
            for t8 in range(L // 128):
                osb = pout.tile([128, DIM], F32, name="osb")
                sl = slice(t8 * 128, (t8 + 1) * 128)
                if last and t8 % 2 == 1:
                    po = ppd.tile([128, LC], F32, tag="ppd", name="ps_o")
                else:
                    po = ppa.tile([128, LC], F32, tag="ppa", name="ps_o")
                nc.tensor.matmul(po[:, 0:DIM], y_sb[:, sl], wouty_sb[:],
                                 start=True, stop=False)
                nc.tensor.matmul(po[:, 0:DIM], xc0[0:DH, sl], woutz_sb[:],
                                 start=False, stop=True)
                nc.scalar.activation(osb[:], po[:, 0:DIM], AF.Copy)
                nc.sync.dma_start(
                    out_d[u, t8 * 128:(t8 + 1) * 128, :], osb[:])

        # ---- software pipeline: staged k-loop (A leads B by 2, B leads C
        # by 1) with FE(u+1) chunks interleaved at fixed steps
        emit_xt_dma(0)
        emit_fe_conv(0)
        # ---- remaining constants ----
        wd1_sb = cp.tile([128, DH], BF)
        nc.gpsimd.dma_start(wd1_sb[:], wd1_d)
        wd2_sb = cp.tile([64, DH], BF)
        nc.gpsimd.dma_start(wd2_sb[:], wd2_d)
        wxpbc1_sb = cp.tile([128, 256], BF)
        nc.gpsimd.dma_start(wxpbc1_sb[:], wxpbc1_d)
        wxpbc2_sb = cp.tile([64, 256], BF)
        nc.gpsimd.dma_start(wxpbc2_sb[:], wxpbc2_d)
        bsp_sb = cp.tile([DH, 1], F32)
        nc.gpsimd.dma_start(bsp_sb[:], bsp_d)
        acol_sb = cp.tile([128, NK], F32)
        nc.gpsimd.dma_start(acol_sb[:], acol_d)
        abcol_sb = cp.tile([128, NK], F32)
        nc.gpsimd.dma_start(abcol_sb[:], abcol_d)
        seli_sb = cp.tile([DH, KP * 128], BF)
        nc.gpsimd.dma_start(seli_sb[:], seli_d)
        seln_sb = cp.tile([128, NK, DH], BF)
        nc.gpsimd.dma_start(seln_sb[:], seln_d.transpose([1, 0, 2]))
        wouty_sb = cp.tile([DH, DIM], BF)
        nc.gpsimd.dma_start(wouty_sb[:], wouty_d)
        woutz_sb = cp.tile([DH, DIM], BF)
        nc.gpsimd.dma_start(woutz_sb[:], woutz_d)

        emit_fe_xproj(0)
        for u in range(NU):
            pys = [ppy.tile([DH, LC], F32, tag=f"pys{lc}",
                            name=f"pys{lc}") for lc in range(NLC)]
            st = {}
            for s in range(NK + 2):
                if s < NK:
                    emit_stage_a(u, s, st)
                if 1 <= s <= NK:
                    emit_stage_b(u, s - 1, st)
                if s >= 2:
                    emit_stage_c(u, s - 2, st, pys)
                if s == 2 and u + 1 < NU:
                    emit_xt_dma(u + 1)
                    with tc.high_priority(200):
                        emit_fe_conv(u + 1)
                if s == 5 and u + 1 < NU:
                    with tc.high_priority(200):
                        emit_fe_xproj(u + 1)
            emit_scan_tail(u, pys, last=(u == NU - 1))

    nc.compile()
    return nc


def _get_nc(ab_same: bool):
    if ab_same not in _NC_CACHE:
        _NC_CACHE[ab_same] = _build(ab_same)
    return _NC_CACHE[ab_same]


def _prep_weights(h, in_proj_w, in_proj_b, conv_w, conv_b, A_log, Ab_log, D,
                  x_proj_w, dt_proj_w, dt_proj_b, out_proj_w):
    import ml_dtypes
    bf = ml_dtypes.bfloat16
    f32 = np.float32
    G = np.arange(96 * h, 96 * h + 96)
    rest = np.array([d for d in range(DIN) if d not in set(G.tolist())])
    perm = np.concatenate([G, rest])

    W_in = in_proj_w.astype(f32)
    M = np.empty((3, DIN, DIM), f32)
    bconv = np.empty((3, DIN), f32)
    for k in range(3):
        M[k] = (conv_w[:, 0, k][:, None] * W_in[0::2, :]
                + conv_w[:, 1, k][:, None] * W_in[1::2, :])
        bconv[k] = (conv_w[:, 0, k] * in_proj_b[0::2]
                    + conv_w[:, 1, k] * in_proj_b[1::2])
    Mp = M[:, perm, :]
    wm = np.empty((9, 128, DIN), f32)
    for s in range(3):
        for kt in range(3):
            wm[s * 3 + kt] = Mp[s][:, kt * 128:(kt + 1) * 128].T
    bias_int = (bconv.sum(0) + conv_b)[perm]
    efix = np.stack([-bconv[0][perm], -bconv[2][perm]])[None]
    bsil = np.zeros((128, 2), f32)
    bsil[:, 0] = bias_int[:128]
    bsil[0:64, 1] = bias_int[128:]

    xp_p = x_proj_w[:, perm].astype(f32)          # (56, 192) permuted
    Wd = dt_proj_w[G].astype(f32) @ xp_p[0:DTR]   # (96, 192)
    wxpbc = np.concatenate(
        [xp_p.T[:, DTR + (np.arange(128) % NST)],
         xp_p.T[:, DTR + NST + (np.arange(128) % NST)]], axis=1)  # (192, 256)

    A = (-np.exp(A_log)).astype(f32)[G]
    Ab = (-np.exp(Ab_log)).astype(f32)[G]
    acol = np.empty((128, NK), f32)
    abcol = np.empty((128, NK), f32)
    r = np.arange(128)
    for k in range(NK):
        acol[:, k] = A[8 * k + r // NST, r % NST]
        abcol[:, k] = Ab[8 * k + r // NST, r % NST]
    seli = np.zeros((DH, KP * 128), f32)
    for k in range(KP):
        seli[8 * k + r // NST, 128 * k + r] = 1.0
    seln = np.zeros((NK, 128, DH), f32)
    for k in range(NK):
        seln[k, r, 8 * k + r // NST] = 1.0

    wouty = out_proj_w[:, G].T.astype(f32)                    # (96, 384)
    woutz = (out_proj_w[:, DIM // 2 + G].T.astype(f32)
             + 2.0 * D[G].astype(f32)[:, None] * wouty)

    return dict(
        wm=wm.astype(bf),
        efix=efix.astype(bf),
        one=np.ones((1, 1), bf),
        bsil=bsil,
        wd1=Wd[:, 0:128].T.astype(bf).copy(),
        wd2=Wd[:, 128:192].T.astype(bf).copy(),
        wxpbc1=wxpbc[0:128].astype(bf).copy(),
        wxpbc2=wxpbc[128:192].astype(bf).copy(),
        bsp=dt_proj_b[G].reshape(DH, 1).astype(f32),
        acol=acol,
        abcol=abcol,
        seli=seli.astype(bf),
        seln=seln.astype(bf),
        wouty=wouty.astype(bf),
        woutz=woutz.astype(bf),
    )


def kernel(x, in_proj_w, in_proj_b, conv_w, conv_b, A_log, Ab_log, D,
           x_proj_w, dt_proj_w, dt_proj_b, out_proj_w, out_proj_b):
    import ml_dtypes
    bf = ml_dtypes.bfloat16
    ab_same = bool(np.array_equal(A_log, Ab_log))
    x = np.asarray(x, np.float32)

    wargs = (in_proj_w, in_proj_b, conv_w, conv_b, A_log, Ab_log, D,
             x_proj_w, dt_proj_w, dt_proj_b, out_proj_w)
    weights = [_prep_weights(h, *[np.asarray(a, np.float32) for a in wargs])
               for h in range(2)]

    xtps = []
    for g in range(B):
        xtp = np.zeros((NU, 3, 128, L + 2), bf)
        for u in range(NU):
            xs = x[g, u * L:(u + 1) * L, :]        # (L, 384)
            xT = np.ascontiguousarray(xs.T)        # (384, L)
            xtp[u, :, :, 1:L + 1] = xT.reshape(3, 128, L).astype(bf)
        xtps.append(xtp)

    in_maps = []
    for core in range(NCORES):
        g, h = divmod(core, 2)
        m = dict(weights[h])
        m["xtp"] = xtps[g]
        in_maps.append(m)

    nc_prog = _get_nc(ab_same)
    r = run_bass_kernel_spmd(nc_prog, in_maps, list(range(NCORES)))
    res = r.results

    out = np.empty((B, SEQ, DIM), np.float32)
    bo = np.asarray(out_proj_b, np.float32)
    for g in range(B):
        for u in range(NU):
            part = (res[2 * g]["out"][u] + res[2 * g + 1]["out"][u] + bo)
            out[g, u * L:(u + 1) * L, :] = part
    return out
